# revision 29
# baseline (speedup 1.0000x reference)
"""Trainium2 Bass kernel for AudioQuantizer (VQ codebook lookup).

Computes, for x [N, 512], codebook [8192, 512], embedding [8192, 512]:
    dist[n,k] = ||x_n||^2 - 2 x_n.c_k + ||c_k||^2
    out[n]    = embedding[argmin_k dist[n,k]]
Sharding: data-parallel over N across 8 cores (codebook replicated).

Device side (per core, n_shard=4096): the PE computes only the cross term
    v[n,k] ~ 2^10 * (2 x_n.c_k)
as an fp8e4m3 DoubleRow matmul sweep: each 128x128 PE cell holds two fp8
weights, so one matmul covers a 256-deep contraction and the whole d=512
reduction takes 2 matmuls per 512-wide k-chunk (vs 4 at fp16) at ~1.8x the
fp16 column rate.  Operands are host-packed: lhsT plane i of partition p
holds x[d = dcp*256 + i*128 + p] (and likewise the codebook), matching the
[Ki, 2, dim] DoubleRow access pattern (validated bit-exact in CoreSim).
Weights stay stationary across the 8 k-chunks of each PSUM half-sweep.

The Act engine evacuates 7 of the 8 double-bank psum tiles per row-tile
into an fp16 band (unshifted: fp8 matmul noise of ~111 band units dwarfs
the fp16 ulp of <=2).  The DVE folds the last psum tile directly against
its band partner with scalar_tensor_tensor(mult, max), then keeps folding
with tensor_tensor(max) -- the only reduction op with the 2x_1p 16-bit
fast path -- so MAX8 + FIND_INDEX8 (1 elem/cycle, no 16-bit speedup) scan
only 512 of the 8192 scores.  A folded slot j stands for the 16 candidates
{j + 512*a}.  Engine balance per 128-row tile: PE ~7.1us, Act ~7.3us,
DVE ~6.8us -- all near-saturated; two-PSUM-operand DVE ops do not compile
(one PSUM port), so Act keeps 7 of 8 psum-tile evacuations.

Host side: fp8 quantization noise is bounded (measured max 111, budget 130
band units; 1 unit = 2^-10 in 2x.c terms), and c_sq spans only ~22 units,
so any slot within ~175 units of the top could win: those slots' 16 alias
candidates (~3 slots/row) are re-scored in f32 and the winner takes the
argmin tie rule.  Rows fall back to the reference's exact fp32 rounding
chain when the winner margin is under the chain+f32 slack, a duplicated
fp16 value makes FIND_INDEX8's first-occurrence indices unreliable, or the
8th folded value is close enough that a non-exported slot could hide a
contender.  ~1k rows flag; validated 0/32768 mismatches on the emulated
pipeline.

Startup: input DMAs are split across the SP and Activation DGE queues (they
serialize per queue), tile-0's weights and first k-chunks live in separate
head tiles, and the bulk codebook pieces are emitted interleaved with
tile-0's matmuls so the first matmul only waits on the head DMAs.

The walrus build here encodes at most one sync-wait per instruction, so
after Tile scheduling we hoist excess waits onto standalone EventSemaphore
instructions (split_multi_waits).
"""

from contextlib import ExitStack

import numpy as np
import ml_dtypes

import concourse.bass as bass
import concourse.mybir as mybir
import concourse.tile as tile
from concourse.bass_utils import run_bass_kernel_spmd

F32 = mybir.dt.float32
F16 = mybir.dt.float16
F8 = mybir.dt.float8e4
U32 = mybir.dt.uint32
FP8 = ml_dtypes.float8_e4m3  # IEEE e4m3 (max 240) -- matches mybir float8e4

P = 128
KC = 512           # k-chunk: psum free dim per matmul group
N_CORES = 8
N_TOTAL = 32768
K_TOTAL = 8192
D = 512

X_SCALE = 4.0      # fp8(x * 2^2): |x| <= ~5.5 -> 22
C_SCALE = 2048.0   # fp8(c * 2^11): |c| <= ~0.055 -> 112 (< 240 cap)
PSUM_SCALE = float(2.0 ** -2)  # 2^12*(2x.c) -> 2^10*(2x.c); no shift: fp8
# noise (~111 units) dwarfs the unshifted fp16 band ulp (<=2 units)
N_FOLD = 4                     # band 8192 -> 512, 16 alias candidates/slot
K_FOLD = K_TOTAL >> N_FOLD


def split_multi_waits(nc, max_waits=1):
    """Hoist excess sync-waits onto standalone EventSemaphore instructions.

    The walrus build here rejects instructions carrying more than one
    sync-wait ("Too many sync wait commands").  Tile attaches several.
    An EventSemaphore on the same engine queue immediately before the
    instruction is semantically equivalent (the queue stalls there).
    """
    n_new = 0
    for f in nc.m.functions:
        for bb in f.blocks:
            insts = list(bb.instructions)
            out = []
            for inst in insts:
                si = inst.sync_info
                waits = list(si.on_wait) if si is not None and si.on_wait else []
                if len(waits) > max_waits:
                    keep = waits[-max_waits:]
                    for i, w in enumerate(waits[:-max_waits]):
                        ev = mybir.InstEventSemaphore(
                            name=f"{inst.name}_hw{i}", ins=[], outs=[]
                        )
                        ev.engine = inst.engine
                        ev.sync_info = mybir.SyncInfo(on_wait=[w], on_update=[])
                        out.append(ev)
                        n_new += 1
                    inst.sync_info = mybir.SyncInfo(
                        on_wait=keep, on_update=list(si.on_update or [])
                    )
                out.append(inst)
            if len(out) != len(insts):
                bb.instructions = out
    return n_new


def build_kernel(n_shard=N_TOTAL // N_CORES, k_total=K_TOTAL, d=D):
    nc = bass.Bass("TRN2", target_bir_lowering=False, debug=False)

    n_tiles = n_shard // P
    n_dcp = d // 256               # DoubleRow d-chunk pairs (contract 256 each)
    k_half = k_total // 2
    assert n_tiles * P == n_shard and n_dcp * 256 == d

    xt_ext = nc.dram_tensor("xdr8", [n_dcp * P, 2 * n_shard], F8, kind="ExternalInput").ap()
    cbt_ext = nc.dram_tensor("cdr8", [n_dcp * P, 2 * k_total], F8, kind="ExternalInput").ap()
    # device-native layout [P, n_tiles*8] (contiguous DMA; host reshapes)
    v8_ext = nc.dram_tensor("v8_out", [P, (n_shard // P) * 8], F16, kind="ExternalOutput").ap()
    i8_ext = nc.dram_tensor("i8_out", [P, (n_shard // P) * 8], U32, kind="ExternalOutput").ap()

    with tile.TileContext(nc) as tc, ExitStack() as ctx:
        consts = ctx.enter_context(tc.tile_pool(name="consts", bufs=1))
        v8a = consts.tile([P, n_tiles * 8], F16, name="v8a")
        i8a = consts.tile([P, n_tiles * 8], U32, name="i8a")

        xt_pool = ctx.enter_context(tc.tile_pool(name="xt", bufs=1))
        cb_pool = ctx.enter_context(tc.tile_pool(name="cb", bufs=1))
        # head tiles: tile-0 weights + k-chunks 0-1, so the first matmuls wait
        # only on these small DMAs (DMA-completion waits are cumulative per
        # queue).  packed column layout per partition: [plane0 | plane1].
        xh = [xt_pool.tile([P, 2 * P], F8, name=f"xh{q}") for q in range(n_dcp)]
        ch = [cb_pool.tile([P, 2 * KC], F8, name=f"ch{q}") for q in range(n_dcp)]
        xdr = [
            xt_pool.tile([P, 2 * n_shard], F8, name=f"xdr{q}") for q in range(n_dcp)
        ]
        cdr = [
            cb_pool.tile([P, 2 * k_total], F8, name=f"cdr{q}") for q in range(n_dcp)
        ]
        xhv = [t[:].rearrange("p (i n) -> p i n", i=2) for t in xh]
        chv = [t[:].rearrange("p (i n) -> p i n", i=2) for t in ch]
        xv = [t[:].rearrange("p (i n) -> p i n", i=2) for t in xdr]
        cv = [t[:].rearrange("p (i n) -> p i n", i=2) for t in cdr]

        # ---- head DMAs (split across the two hwdge queues: SP + Act) ----
        for q in range(n_dcp):
            rs = slice(q * P, (q + 1) * P)
            nc.sync.dma_start(xh[q][:, 0:P], xt_ext[rs, 0:P])
            nc.scalar.dma_start(xh[q][:, P : 2 * P], xt_ext[rs, n_shard : n_shard + P])
            nc.sync.dma_start(ch[q][:, 0:KC], cbt_ext[rs, 0:KC])
            nc.scalar.dma_start(
                ch[q][:, KC : 2 * KC], cbt_ext[rs, k_total : k_total + KC]
            )

        def cb_piece(c0, c1):  # k-chunks [c0, c1): both planes, both dcp
            for q in range(n_dcp):
                rs = slice(q * P, (q + 1) * P)
                nc.sync.dma_start(
                    cdr[q][:, c0 * KC : c1 * KC], cbt_ext[rs, c0 * KC : c1 * KC]
                )
                nc.scalar.dma_start(
                    cdr[q][:, k_total + c0 * KC : k_total + c1 * KC],
                    cbt_ext[rs, k_total + c0 * KC : k_total + c1 * KC],
                )

        def xt_piece(lo, hi):  # x columns [lo, hi): both planes, both dcp
            for q in range(n_dcp):
                rs = slice(q * P, (q + 1) * P)
                nc.sync.dma_start(xdr[q][:, lo:hi], xt_ext[rs, lo:hi])
                nc.scalar.dma_start(
                    xdr[q][:, n_shard + lo : n_shard + hi],
                    xt_ext[rs, n_shard + lo : n_shard + hi],
                )

        band_pool = ctx.enter_context(tc.tile_pool(name="band", bufs=3))
        fold_pool = ctx.enter_context(tc.tile_pool(name="fold", bufs=3))
        mm_psum = ctx.enter_context(tc.tile_pool(name="mmps", bufs=4, space="PSUM"))

        for t in range(n_tiles):
            band = band_pool.tile([P, 14 * KC], F16, tag="band")
            t1 = fold_pool.tile([P, k_half], F16, tag="t1")
            for h in range(2):
                pst = [
                    mm_psum.tile([P, 2 * KC], F32, tag="mm", name=f"mm{q}")
                    for q in range(4)
                ]
                for dcp in range(n_dcp):
                    for c in range(8):
                        kc = h * 8 + c
                        # bulk codebook pieces must be EMITTED before their
                        # first reader (program order defines RAW deps), but
                        # after the head-chunk matmuls so those only wait on
                        # the head DMAs.
                        if t == 0 and h == 0 and dcp == 0 and c == 1:
                            cb_piece(1, 8)
                        if t == 0 and kc >= 1:
                            lhs = xhv[dcp][:, :, 0:P]
                            rhs = cv[dcp][:, :, kc * KC : (kc + 1) * KC]
                        elif t == 0:
                            lhs = xhv[dcp][:, :, 0:P]
                            rhs = chv[dcp][:, :, kc * KC : (kc + 1) * KC]
                        else:
                            lhs = xv[dcp][:, :, t * P : (t + 1) * P]
                            rhs = (
                                chv[dcp][:, :, 0:KC]
                                if kc < 1
                                else cv[dcp][:, :, kc * KC : (kc + 1) * KC]
                            )
                        nc.tensor.matmul(
                            pst[c // 2][:, (c % 2) * KC : (c % 2 + 1) * KC],
                            lhs,
                            rhs,
                            start=(dcp == 0),
                            stop=(dcp == n_dcp - 1),
                            perf_mode=mybir.MatmulPerfMode.DoubleRow,
                            skip_group_check=True,
                        )
                for q in range(4):
                    k0 = h * 8 * KC + q * 2 * KC
                    if h == 1 and q == 3:
                        # chunks 14-15: DVE folds psum straight into t1
                        # (k j+4096 vs band k j for j in [3072, 4096))
                        nc.vector.scalar_tensor_tensor(
                            t1[:, 3072:4096],
                            pst[q][:],
                            float(PSUM_SCALE),
                            band[:, 3072:4096],
                            op0=mybir.AluOpType.mult,
                            op1=mybir.AluOpType.max,
                        )
                    elif h == 1 and q == 2 and t % 2 == 0:
                        # alternate tiles: DVE takes 512 of this evac so the
                        # Act engine (the pacer) matches the DVE load
                        nc.scalar.mul(band[:, k0 : k0 + KC], pst[q][:, 0:KC], PSUM_SCALE)
                        nc.vector.tensor_scalar_mul(
                            band[:, k0 + KC : k0 + 2 * KC],
                            pst[q][:, KC : 2 * KC],
                            float(PSUM_SCALE),
                        )
                        nc.vector.tensor_tensor(
                            out=t1[:, q * 1024 : (q + 1) * 1024],
                            in0=band[:, q * 1024 : (q + 1) * 1024],
                            in1=band[:, k0 : k0 + 2 * KC],
                            op=mybir.AluOpType.max,
                        )
                    else:
                        nc.scalar.mul(band[:, k0 : k0 + 2 * KC], pst[q][:], PSUM_SCALE)
                        if h == 1:
                            # pipeline fold1a piece q right behind its evac
                            nc.vector.tensor_tensor(
                                out=t1[:, q * 1024 : (q + 1) * 1024],
                                in0=band[:, q * 1024 : (q + 1) * 1024],
                                in1=band[:, k0 : k0 + 2 * KC],
                                op=mybir.AluOpType.max,
                            )
                if t == 0 and h == 0:
                    cb_piece(8, 16)
            if t == 0:
                xt_piece(P, n_shard)

            t2 = fold_pool.tile([P, k_half // 2], F16, tag="t2")
            nc.vector.tensor_tensor(
                out=t2[:],
                in0=t1[:, 0 : k_half // 2],
                in1=t1[:, k_half // 2 : k_half],
                op=mybir.AluOpType.max,
            )
            t3 = fold_pool.tile([P, 2 * K_FOLD], F16, tag="t3")
            nc.vector.tensor_tensor(
                out=t3[:],
                in0=t2[:, 0 : 2 * K_FOLD],
                in1=t2[:, 2 * K_FOLD : k_half // 2],
                op=mybir.AluOpType.max,
            )
            t4 = fold_pool.tile([P, K_FOLD], F16, tag="t4")
            nc.vector.tensor_tensor(
                out=t4[:],
                in0=t3[:, 0:K_FOLD],
                in1=t3[:, K_FOLD : 2 * K_FOLD],
                op=mybir.AluOpType.max,
            )
            v8s = v8a[:, t * 8 : (t + 1) * 8]
            nc.vector.max(v8s, t4[:])
            nc.vector.max_index(i8a[:, t * 8 : (t + 1) * 8], v8s, t4[:])

        nc.sync.dma_start(v8_ext, v8a[:])
        nc.sync.dma_start(i8_ext, i8a[:])

    return nc


_NC_CACHE = {}


def _get_nc():
    if "nc" not in _NC_CACHE:
        nc = build_kernel()
        split_multi_waits(nc)
        _NC_CACHE["nc"] = nc
    return _NC_CACHE["nc"]


def _pack_dr(arrT):
    """[d, cols] -> DoubleRow-packed [n_dcp*128, 2*cols] (plane-major)."""
    d = arrT.shape[0]
    out = []
    for dcp in range(d // 256):
        pl = arrT[dcp * 256 : (dcp + 1) * 256]          # [256, cols]
        out.append(
            np.ascontiguousarray(
                np.stack([pl[0:P], pl[P : 2 * P]], axis=1).reshape(P, -1)
            )
        )
    return np.concatenate(out, axis=0)


# ---------------- host side ----------------

# band-unit error budget (1 unit = 2^-10 raw 2x.c):
E_MM = 130.0        # fp8 matmul quantization noise hard ceiling (measured max 111)
SEL_NOISE = 150.0   # selection-window noise allowance (~6 sigma of error diff)
CHAIN_SLACK = 3e-4  # reference fp32 rounding-chain slack, raw units
MARGIN_THR = 4e-4   # raw-unit winner margin below which we replay the chain
N_ALIAS = 1 << N_FOLD


def _host_decide(x, codebook, v8, i8):
    """Resolve folded top-8 candidates; return (idx, flagged_rows)."""
    n = x.shape[0]
    cb64 = codebook.astype(np.float64)
    csq64 = np.einsum("kd,kd->k", cb64, cb64)
    csq_min = csq64.min()
    csq_range = csq64.max() - csq_min
    csq32 = csq64.astype(np.float32)

    v8f = v8.astype(np.float32)
    # per-value device-vs-true bound in band units: fp8 noise + fp16 half-ulp
    e_val = (E_MM + 0.5 * np.spacing(np.abs(v8))).astype(np.float32)
    # window: slots whose true max-alias score could plausibly win after csq
    W = csq_range * 1024.0 + SEL_NOISE + CHAIN_SLACK * 1024.0
    sel = (v8f[:, 0:1] - v8f) <= W        # [n, 8], always includes slot 0

    rr, ss = np.nonzero(sel)
    jj = i8[rr, ss].astype(np.int64)      # folded index in [0, K_FOLD)
    xs = x[rr]                            # [m, 512] f32
    score = np.empty((len(rr), N_ALIAS), dtype=np.float64)
    kk = np.empty((len(rr), N_ALIAS), dtype=np.int64)
    for a in range(N_ALIAS):
        ka = jj + a * K_FOLD
        kk[:, a] = ka
        score[:, a] = 2.0 * np.einsum("md,md->m", xs, codebook[ka]) - csq32[ka]

    # winner per row: max score, ties -> lowest k
    flat_r = np.repeat(rr, N_ALIAS)
    flat_s = score.reshape(-1)
    flat_k = kk.reshape(-1)
    order = np.lexsort((flat_k, -flat_s, flat_r))
    fr, fs, fk = flat_r[order], flat_s[order], flat_k[order]
    first = np.r_[True, fr[1:] != fr[:-1]]
    win_rows = fr[first]
    idx = np.zeros(n, dtype=np.int64)
    win_score = np.zeros(n, dtype=np.float64)
    runner = np.full(n, -np.inf)
    idx[win_rows] = fk[first]
    win_score[win_rows] = fs[first]
    pos = np.nonzero(first)[0]
    has2 = np.r_[pos[1:], len(fr)] - pos >= 2
    runner[win_rows[has2]] = fs[pos[has2] + 1]

    # flags (margin widened for the f32 resolve's own rounding)
    margin_flag = (win_score - runner) < MARGIN_THR
    hidden_ub = (v8f[:, 7] + e_val[:, 7]) * (2.0 ** -10) - csq_min
    hidden_flag = win_score < hidden_ub + CHAIN_SLACK
    dup_in_w = np.any((v8[:, :-1] == v8[:, 1:]) & sel[:, 1:], axis=1)
    flagged = np.nonzero(margin_flag | hidden_flag | dup_in_w)[0]
    return idx, flagged


def _exact_chain_rows(x, codebook, rows):
    """Reference's exact fp32 rounding chain for the given rows (f64 math)."""
    x64 = x[rows].astype(np.float64)
    cb64 = codebook.astype(np.float64)
    xsq32 = np.einsum("md,md->m", x64, x64).astype(np.float32)
    csq32 = np.einsum("kd,kd->k", cb64, cb64).astype(np.float32)
    cr32 = (2.0 * (x64 @ cb64.T)).astype(np.float32)
    d1 = (xsq32[:, None].astype(np.float64) - cr32.astype(np.float64)).astype(np.float32)
    d2 = (d1.astype(np.float64) + csq32.astype(np.float64)[None, :]).astype(np.float32)
    return np.argmin(d2, axis=1).astype(np.int64)


def kernel(x, codebook, embedding, **run_kwargs):
    x = np.ascontiguousarray(np.asarray(x, dtype=np.float32))
    codebook = np.ascontiguousarray(np.asarray(codebook, dtype=np.float32))
    embedding = np.ascontiguousarray(np.asarray(embedding, dtype=np.float32))
    n = x.shape[0]
    n_shard = n // N_CORES
    nc = _get_nc()

    xq8 = (x.T * np.float32(X_SCALE)).astype(FP8)         # [512, n]
    cq8 = (codebook.T * np.float32(C_SCALE)).astype(FP8)  # [512, 8192]
    cdr8 = _pack_dr(cq8)                                  # [256, 2*8192]
    xdr8_full = _pack_dr(xq8)                             # [256, 2*n]
    in_maps = []
    for i in range(N_CORES):
        sl = xdr8_full.reshape(2 * P, 2, n)[:, :, i * n_shard : (i + 1) * n_shard]
        in_maps.append(
            {
                "xdr8": np.ascontiguousarray(sl.reshape(2 * P, 2 * n_shard)),
                "cdr8": cdr8,
            }
        )
    res = run_bass_kernel_spmd(nc, in_maps, core_ids=list(range(N_CORES)), **run_kwargs)

    def unpack(name, dt):
        # [P, n_tiles*8] device layout -> [n_shard, 8]: row = t*P + p
        return np.concatenate(
            [
                np.ascontiguousarray(
                    res.results[i][name]
                    .reshape(P, n_shard // P, 8)
                    .transpose(1, 0, 2)
                ).reshape(n_shard, 8)
                for i in range(N_CORES)
            ],
            axis=0,
        )

    v8 = unpack("v8_out", np.float16)
    i8 = unpack("i8_out", np.uint32)
    kernel.last_results = res

    idx, flagged = _host_decide(x, codebook, v8, i8)
    if flagged.size:
        idx[flagged] = _exact_chain_rows(x, codebook, flagged)
    kernel.n_flagged = len(flagged)
    return embedding[idx]


# revision 30
# speedup vs baseline: 1.0049x; 1.0049x over previous
"""Trainium2 Bass kernel for AudioQuantizer (VQ codebook lookup).

Computes, for x [N, 512], codebook [8192, 512], embedding [8192, 512]:
    dist[n,k] = ||x_n||^2 - 2 x_n.c_k + ||c_k||^2
    out[n]    = embedding[argmin_k dist[n,k]]
Sharding: data-parallel over N across 8 cores (codebook replicated).

Device side (per core, n_shard=4096): the PE computes only the cross term
    v[n,k] ~ 2^10 * (2 x_n.c_k)
as an fp8e4m3 DoubleRow matmul sweep: each 128x128 PE cell holds two fp8
weights, so one matmul covers a 256-deep contraction and the whole d=512
reduction takes 2 matmuls per 512-wide k-chunk (vs 4 at fp16) at ~1.8x the
fp16 column rate.  Operands are host-packed: lhsT plane i of partition p
holds x[d = dcp*256 + i*128 + p] (and likewise the codebook), matching the
[Ki, 2, dim] DoubleRow access pattern (validated bit-exact in CoreSim).
Weights stay stationary across the 8 k-chunks of each PSUM half-sweep.

The Act engine evacuates 7 of the 8 double-bank psum tiles per row-tile
into an fp16 band (unshifted: fp8 matmul noise of ~111 band units dwarfs
the fp16 ulp of <=2).  The DVE folds the last psum tile directly against
its band partner with scalar_tensor_tensor(mult, max), then keeps folding
with tensor_tensor(max) -- the only reduction op with the 2x_1p 16-bit
fast path -- so MAX8 + FIND_INDEX8 (1 elem/cycle, no 16-bit speedup) scan
only 512 of the 8192 scores.  A folded slot j stands for the 16 candidates
{j + 512*a}.  Engine balance per 128-row tile: PE ~7.1us, Act ~7.3us,
DVE ~6.8us -- all near-saturated; two-PSUM-operand DVE ops do not compile
(one PSUM port), so Act keeps 7 of 8 psum-tile evacuations.

Host side: fp8 quantization noise is bounded (measured max 111, budget 130
band units; 1 unit = 2^-10 in 2x.c terms), and c_sq spans only ~22 units,
so any slot within ~175 units of the top could win: those slots' 16 alias
candidates (~3 slots/row) are re-scored in f32 and the winner takes the
argmin tie rule.  Rows fall back to the reference's exact fp32 rounding
chain when the winner margin is under the chain+f32 slack, a duplicated
fp16 value makes FIND_INDEX8's first-occurrence indices unreliable, or the
8th folded value is close enough that a non-exported slot could hide a
contender.  ~1k rows flag; validated 0/32768 mismatches on the emulated
pipeline.

Startup: input DMAs are split across the SP and Activation DGE queues (they
serialize per queue), tile-0's weights and first k-chunks live in separate
head tiles, and the bulk codebook pieces are emitted interleaved with
tile-0's matmuls so the first matmul only waits on the head DMAs.

The walrus build here encodes at most one sync-wait per instruction, so
after Tile scheduling we hoist excess waits onto standalone EventSemaphore
instructions (split_multi_waits).
"""

from contextlib import ExitStack

import numpy as np
import ml_dtypes

import concourse.bass as bass
import concourse.mybir as mybir
import concourse.tile as tile
from concourse.bass_utils import run_bass_kernel_spmd

F32 = mybir.dt.float32
F16 = mybir.dt.float16
F8 = mybir.dt.float8e4
U32 = mybir.dt.uint32
FP8 = ml_dtypes.float8_e4m3  # IEEE e4m3 (max 240) -- matches mybir float8e4

P = 128
KC = 512           # k-chunk: psum free dim per matmul group
N_CORES = 8
N_TOTAL = 32768
K_TOTAL = 8192
D = 512

X_SCALE = 4.0      # fp8(x * 2^2): |x| <= ~5.5 -> 22
C_SCALE = 2048.0   # fp8(c * 2^11): |c| <= ~0.055 -> 112 (< 240 cap)
PSUM_SCALE = float(2.0 ** -2)  # 2^12*(2x.c) -> 2^10*(2x.c); no shift: fp8
# noise (~111 units) dwarfs the unshifted fp16 band ulp (<=2 units)
N_FOLD = 4                     # band 8192 -> 512, 16 alias candidates/slot
K_FOLD = K_TOTAL >> N_FOLD


def split_multi_waits(nc, max_waits=1):
    """Hoist excess sync-waits onto standalone EventSemaphore instructions.

    The walrus build here rejects instructions carrying more than one
    sync-wait ("Too many sync wait commands").  Tile attaches several.
    An EventSemaphore on the same engine queue immediately before the
    instruction is semantically equivalent (the queue stalls there).
    """
    n_new = 0
    for f in nc.m.functions:
        for bb in f.blocks:
            insts = list(bb.instructions)
            out = []
            for inst in insts:
                si = inst.sync_info
                waits = list(si.on_wait) if si is not None and si.on_wait else []
                if len(waits) > max_waits:
                    keep = waits[-max_waits:]
                    for i, w in enumerate(waits[:-max_waits]):
                        ev = mybir.InstEventSemaphore(
                            name=f"{inst.name}_hw{i}", ins=[], outs=[]
                        )
                        ev.engine = inst.engine
                        ev.sync_info = mybir.SyncInfo(on_wait=[w], on_update=[])
                        out.append(ev)
                        n_new += 1
                    inst.sync_info = mybir.SyncInfo(
                        on_wait=keep, on_update=list(si.on_update or [])
                    )
                out.append(inst)
            if len(out) != len(insts):
                bb.instructions = out
    return n_new


def build_kernel(n_shard=N_TOTAL // N_CORES, k_total=K_TOTAL, d=D):
    nc = bass.Bass("TRN2", target_bir_lowering=False, debug=False)

    n_tiles = n_shard // P
    n_dcp = d // 256               # DoubleRow d-chunk pairs (contract 256 each)
    k_half = k_total // 2
    assert n_tiles * P == n_shard and n_dcp * 256 == d

    xt_ext = nc.dram_tensor("xdr8", [n_dcp * P, 2 * n_shard], F8, kind="ExternalInput").ap()
    cbt_ext = nc.dram_tensor("cdr8", [n_dcp * P, 2 * k_total], F8, kind="ExternalInput").ap()
    # device-native layout [P, n_tiles*8] (contiguous DMA; host reshapes)
    v8_ext = nc.dram_tensor("v8_out", [P, (n_shard // P) * 8], F16, kind="ExternalOutput").ap()
    i8_ext = nc.dram_tensor("i8_out", [P, (n_shard // P) * 8], U32, kind="ExternalOutput").ap()

    with tile.TileContext(nc) as tc, ExitStack() as ctx:
        consts = ctx.enter_context(tc.tile_pool(name="consts", bufs=1))
        v8a = consts.tile([P, n_tiles * 8], F16, name="v8a")
        i8a = consts.tile([P, n_tiles * 8], U32, name="i8a")

        xt_pool = ctx.enter_context(tc.tile_pool(name="xt", bufs=1))
        cb_pool = ctx.enter_context(tc.tile_pool(name="cb", bufs=1))
        # head tiles: tile-0 weights + k-chunks 0-1, so the first matmuls wait
        # only on these small DMAs (DMA-completion waits are cumulative per
        # queue).  packed column layout per partition: [plane0 | plane1].
        xh = [xt_pool.tile([P, 2 * P], F8, name=f"xh{q}") for q in range(n_dcp)]
        ch = [cb_pool.tile([P, 2 * KC], F8, name=f"ch{q}") for q in range(n_dcp)]
        xdr = [
            xt_pool.tile([P, 2 * n_shard], F8, name=f"xdr{q}") for q in range(n_dcp)
        ]
        cdr = [
            cb_pool.tile([P, 2 * k_total], F8, name=f"cdr{q}") for q in range(n_dcp)
        ]
        xhv = [t[:].rearrange("p (i n) -> p i n", i=2) for t in xh]
        chv = [t[:].rearrange("p (i n) -> p i n", i=2) for t in ch]
        xv = [t[:].rearrange("p (i n) -> p i n", i=2) for t in xdr]
        cv = [t[:].rearrange("p (i n) -> p i n", i=2) for t in cdr]

        # ---- head DMAs (split across the two hwdge queues: SP + Act) ----
        for q in range(n_dcp):
            rs = slice(q * P, (q + 1) * P)
            nc.sync.dma_start(xh[q][:, 0:P], xt_ext[rs, 0:P])
            nc.scalar.dma_start(xh[q][:, P : 2 * P], xt_ext[rs, n_shard : n_shard + P])
            nc.sync.dma_start(ch[q][:, 0:KC], cbt_ext[rs, 0:KC])
            nc.scalar.dma_start(
                ch[q][:, KC : 2 * KC], cbt_ext[rs, k_total : k_total + KC]
            )

        def cb_piece(c0, c1):  # k-chunks [c0, c1): both planes, both dcp
            for q in range(n_dcp):
                rs = slice(q * P, (q + 1) * P)
                nc.sync.dma_start(
                    cdr[q][:, c0 * KC : c1 * KC], cbt_ext[rs, c0 * KC : c1 * KC]
                )
                nc.scalar.dma_start(
                    cdr[q][:, k_total + c0 * KC : k_total + c1 * KC],
                    cbt_ext[rs, k_total + c0 * KC : k_total + c1 * KC],
                )

        def xt_piece(lo, hi):  # x columns [lo, hi): both planes, both dcp
            for q in range(n_dcp):
                rs = slice(q * P, (q + 1) * P)
                nc.sync.dma_start(xdr[q][:, lo:hi], xt_ext[rs, lo:hi])
                nc.scalar.dma_start(
                    xdr[q][:, n_shard + lo : n_shard + hi],
                    xt_ext[rs, n_shard + lo : n_shard + hi],
                )

        band_pool = ctx.enter_context(tc.tile_pool(name="band", bufs=3))
        fold_pool = ctx.enter_context(tc.tile_pool(name="fold", bufs=3))
        mm_psum = ctx.enter_context(tc.tile_pool(name="mmps", bufs=4, space="PSUM"))

        for t in range(n_tiles):
            band = band_pool.tile([P, 14 * KC], F16, tag="band")
            t1 = fold_pool.tile([P, k_half], F16, tag="t1")
            for h in range(2):
                pst = [
                    mm_psum.tile([P, 2 * KC], F32, tag="mm", name=f"mm{q}")
                    for q in range(4)
                ]
                for dcp in range(n_dcp):
                    for c in range(8):
                        kc = h * 8 + c
                        # bulk codebook pieces must be EMITTED before their
                        # first reader (program order defines RAW deps), but
                        # after the head-chunk matmuls so those only wait on
                        # the head DMAs.
                        if t == 0 and h == 0 and dcp == 0 and c == 1:
                            cb_piece(1, 8)
                        if t == 0 and kc >= 1:
                            lhs = xhv[dcp][:, :, 0:P]
                            rhs = cv[dcp][:, :, kc * KC : (kc + 1) * KC]
                        elif t == 0:
                            lhs = xhv[dcp][:, :, 0:P]
                            rhs = chv[dcp][:, :, kc * KC : (kc + 1) * KC]
                        else:
                            lhs = xv[dcp][:, :, t * P : (t + 1) * P]
                            rhs = (
                                chv[dcp][:, :, 0:KC]
                                if kc < 1
                                else cv[dcp][:, :, kc * KC : (kc + 1) * KC]
                            )
                        nc.tensor.matmul(
                            pst[c // 2][:, (c % 2) * KC : (c % 2 + 1) * KC],
                            lhs,
                            rhs,
                            start=(dcp == 0),
                            stop=(dcp == n_dcp - 1),
                            perf_mode=mybir.MatmulPerfMode.DoubleRow,
                            skip_group_check=True,
                        )
                for q in range(4):
                    k0 = h * 8 * KC + q * 2 * KC
                    if h == 1 and q == 3:
                        # chunks 14-15: DVE folds psum straight into t1
                        # (k j+4096 vs band k j for j in [3072, 4096))
                        nc.vector.scalar_tensor_tensor(
                            t1[:, 3072:4096],
                            pst[q][:],
                            float(PSUM_SCALE),
                            band[:, 3072:4096],
                            op0=mybir.AluOpType.mult,
                            op1=mybir.AluOpType.max,
                        )
                    else:
                        nc.scalar.mul(band[:, k0 : k0 + 2 * KC], pst[q][:], PSUM_SCALE)
                        if h == 1:
                            # pipeline fold1a piece q right behind its evac
                            nc.vector.tensor_tensor(
                                out=t1[:, q * 1024 : (q + 1) * 1024],
                                in0=band[:, q * 1024 : (q + 1) * 1024],
                                in1=band[:, k0 : k0 + 2 * KC],
                                op=mybir.AluOpType.max,
                            )
                if t == 0 and h == 0:
                    cb_piece(8, 16)
            if t == 0:
                xt_piece(P, n_shard)

            t2 = fold_pool.tile([P, k_half // 2], F16, tag="t2")
            nc.vector.tensor_tensor(
                out=t2[:],
                in0=t1[:, 0 : k_half // 2],
                in1=t1[:, k_half // 2 : k_half],
                op=mybir.AluOpType.max,
            )
            t3 = fold_pool.tile([P, 2 * K_FOLD], F16, tag="t3")
            nc.vector.tensor_tensor(
                out=t3[:],
                in0=t2[:, 0 : 2 * K_FOLD],
                in1=t2[:, 2 * K_FOLD : k_half // 2],
                op=mybir.AluOpType.max,
            )
            t4 = fold_pool.tile([P, K_FOLD], F16, tag="t4")
            nc.vector.tensor_tensor(
                out=t4[:],
                in0=t3[:, 0:K_FOLD],
                in1=t3[:, K_FOLD : 2 * K_FOLD],
                op=mybir.AluOpType.max,
            )
            v8s = v8a[:, t * 8 : (t + 1) * 8]
            nc.vector.max(v8s, t4[:])
            nc.vector.max_index(i8a[:, t * 8 : (t + 1) * 8], v8s, t4[:])

        nc.sync.dma_start(v8_ext, v8a[:])
        nc.sync.dma_start(i8_ext, i8a[:])

    return nc


_NC_CACHE = {}


def _get_nc():
    if "nc" not in _NC_CACHE:
        nc = build_kernel()
        split_multi_waits(nc)
        _NC_CACHE["nc"] = nc
    return _NC_CACHE["nc"]


def _pack_dr(arrT):
    """[d, cols] -> DoubleRow-packed [n_dcp*128, 2*cols] (plane-major)."""
    d = arrT.shape[0]
    out = []
    for dcp in range(d // 256):
        pl = arrT[dcp * 256 : (dcp + 1) * 256]          # [256, cols]
        out.append(
            np.ascontiguousarray(
                np.stack([pl[0:P], pl[P : 2 * P]], axis=1).reshape(P, -1)
            )
        )
    return np.concatenate(out, axis=0)


# ---------------- host side ----------------

# band-unit error budget (1 unit = 2^-10 raw 2x.c):
E_MM = 130.0        # fp8 matmul quantization noise hard ceiling (measured max 111)
SEL_NOISE = 150.0   # selection-window noise allowance (~6 sigma of error diff)
CHAIN_SLACK = 3e-4  # reference fp32 rounding-chain slack, raw units
MARGIN_THR = 4e-4   # raw-unit winner margin below which we replay the chain
N_ALIAS = 1 << N_FOLD


def _host_decide(x, codebook, v8, i8):
    """Resolve folded top-8 candidates; return (idx, flagged_rows)."""
    n = x.shape[0]
    cb64 = codebook.astype(np.float64)
    csq64 = np.einsum("kd,kd->k", cb64, cb64)
    csq_min = csq64.min()
    csq_range = csq64.max() - csq_min
    csq32 = csq64.astype(np.float32)

    v8f = v8.astype(np.float32)
    # per-value device-vs-true bound in band units: fp8 noise + fp16 half-ulp
    e_val = (E_MM + 0.5 * np.spacing(np.abs(v8))).astype(np.float32)
    # window: slots whose true max-alias score could plausibly win after csq
    W = csq_range * 1024.0 + SEL_NOISE + CHAIN_SLACK * 1024.0
    sel = (v8f[:, 0:1] - v8f) <= W        # [n, 8], always includes slot 0

    rr, ss = np.nonzero(sel)
    jj = i8[rr, ss].astype(np.int64)      # folded index in [0, K_FOLD)
    xs = x[rr]                            # [m, 512] f32
    score = np.empty((len(rr), N_ALIAS), dtype=np.float64)
    kk = np.empty((len(rr), N_ALIAS), dtype=np.int64)
    for a in range(N_ALIAS):
        ka = jj + a * K_FOLD
        kk[:, a] = ka
        score[:, a] = 2.0 * np.einsum("md,md->m", xs, codebook[ka]) - csq32[ka]

    # winner per row: max score, ties -> lowest k
    flat_r = np.repeat(rr, N_ALIAS)
    flat_s = score.reshape(-1)
    flat_k = kk.reshape(-1)
    order = np.lexsort((flat_k, -flat_s, flat_r))
    fr, fs, fk = flat_r[order], flat_s[order], flat_k[order]
    first = np.r_[True, fr[1:] != fr[:-1]]
    win_rows = fr[first]
    idx = np.zeros(n, dtype=np.int64)
    win_score = np.zeros(n, dtype=np.float64)
    runner = np.full(n, -np.inf)
    idx[win_rows] = fk[first]
    win_score[win_rows] = fs[first]
    pos = np.nonzero(first)[0]
    has2 = np.r_[pos[1:], len(fr)] - pos >= 2
    runner[win_rows[has2]] = fs[pos[has2] + 1]

    # flags (margin widened for the f32 resolve's own rounding)
    margin_flag = (win_score - runner) < MARGIN_THR
    hidden_ub = (v8f[:, 7] + e_val[:, 7]) * (2.0 ** -10) - csq_min
    hidden_flag = win_score < hidden_ub + CHAIN_SLACK
    dup_in_w = np.any((v8[:, :-1] == v8[:, 1:]) & sel[:, 1:], axis=1)
    flagged = np.nonzero(margin_flag | hidden_flag | dup_in_w)[0]
    return idx, flagged


def _exact_chain_rows(x, codebook, rows):
    """Reference's exact fp32 rounding chain for the given rows (f64 math)."""
    x64 = x[rows].astype(np.float64)
    cb64 = codebook.astype(np.float64)
    xsq32 = np.einsum("md,md->m", x64, x64).astype(np.float32)
    csq32 = np.einsum("kd,kd->k", cb64, cb64).astype(np.float32)
    cr32 = (2.0 * (x64 @ cb64.T)).astype(np.float32)
    d1 = (xsq32[:, None].astype(np.float64) - cr32.astype(np.float64)).astype(np.float32)
    d2 = (d1.astype(np.float64) + csq32.astype(np.float64)[None, :]).astype(np.float32)
    return np.argmin(d2, axis=1).astype(np.int64)


def kernel(x, codebook, embedding, **run_kwargs):
    x = np.ascontiguousarray(np.asarray(x, dtype=np.float32))
    codebook = np.ascontiguousarray(np.asarray(codebook, dtype=np.float32))
    embedding = np.ascontiguousarray(np.asarray(embedding, dtype=np.float32))
    n = x.shape[0]
    n_shard = n // N_CORES
    nc = _get_nc()

    xq8 = (x.T * np.float32(X_SCALE)).astype(FP8)         # [512, n]
    cq8 = (codebook.T * np.float32(C_SCALE)).astype(FP8)  # [512, 8192]
    cdr8 = _pack_dr(cq8)                                  # [256, 2*8192]
    xdr8_full = _pack_dr(xq8)                             # [256, 2*n]
    in_maps = []
    for i in range(N_CORES):
        sl = xdr8_full.reshape(2 * P, 2, n)[:, :, i * n_shard : (i + 1) * n_shard]
        in_maps.append(
            {
                "xdr8": np.ascontiguousarray(sl.reshape(2 * P, 2 * n_shard)),
                "cdr8": cdr8,
            }
        )
    res = run_bass_kernel_spmd(nc, in_maps, core_ids=list(range(N_CORES)), **run_kwargs)

    def unpack(name, dt):
        # [P, n_tiles*8] device layout -> [n_shard, 8]: row = t*P + p
        return np.concatenate(
            [
                np.ascontiguousarray(
                    res.results[i][name]
                    .reshape(P, n_shard // P, 8)
                    .transpose(1, 0, 2)
                ).reshape(n_shard, 8)
                for i in range(N_CORES)
            ],
            axis=0,
        )

    v8 = unpack("v8_out", np.float16)
    i8 = unpack("i8_out", np.uint32)
    kernel.last_results = res

    idx, flagged = _host_decide(x, codebook, v8, i8)
    if flagged.size:
        idx[flagged] = _exact_chain_rows(x, codebook, flagged)
    kernel.n_flagged = len(flagged)
    return embedding[idx]
